# revision 19
# baseline (speedup 1.0000x reference)
"""Causal shaped attention kernel for Trainium2 (8 NeuronCores).

y = beta * softmax(causal(q k^T / 8)) @ v + alpha * Id @ v - gamma * MC @ v
  with q,k = x @ w_attn.T split, v = x, Id = softmax(eye(T)), MC = causal row-mean.

Sharding: (batch, head-group) across 8 cores: core c -> b = c//2, heads
h0 = (c%2)*8 .. h0+8.  Each core computes y[b, :, h0*64 : h0*64+512].

Id@v and MC@v have closed forms (no T x T materialization):
  Id@v[i] = ((e-1) v[i] + colsum(v)) / (e+T-1)
  MC@v[i] = cumsum(v)[i] / (i+1)

On-device layout (per core):
  xT   [128, 8, 2048]   x[b]^T by 128-wide c-chunks (PE-transposed)
  WTq  [128, 4, 8, 128] per head-pair p, c-chunk ci: [Wq_even^T | Wq_odd^T]
  WTk  same for k
  qkT  [128, 4, 2, 2048] pair p: partitions 0:64 even head, 64:128 odd head;
                         [.., 0, :] = q^T, [.., 1, :] = k^T
  vones [128, 8, 16, 65] per head hh, j-tile J: [v | 1]  (AV lhsT)
  static [128, 16, 512]  k1*v + k2*colsum - gamma*cumsum/(i+1), combine addend

Attention per (head, i-strip g of 512): S^T tiles [j=128, i<=512] via PE
(K=64, fp32r), exp on ACT (scale=1/8 folded in), causal diag masked by tril
multiply, AV matmul lhsT=[v|1] gives y^T and rowsum in one pass, PE transpose
back, normalize + add static, DMA out.
"""

import sys

if "/opt/trn_rl_repo" not in sys.path:
    sys.path.insert(0, "/opt/trn_rl_repo")

import math

import numpy as np

import concourse.bass as bass
import concourse.mybir as mybir
import concourse.tile as tile
from concourse import bacc
from concourse.bass_utils import run_bass_kernel_spmd

F32 = mybir.dt.float32
F32R = mybir.dt.float32r
AF = mybir.ActivationFunctionType
OP = mybir.AluOpType

N_CORES = 8
B, T, C = 4, 2048, 1024
H, HD = 16, 64
NHC = 8          # heads per core
NT = T // 128    # 16 j/i tiles
NS = 4           # i-strips of 512
CONSTS_W = 276   # 128 tril + 16 negipg + k1 + k2 + kb + pad + 128 ident

_NC_CACHE = {}


def r(ap):
    return ap.bitcast(F32R)


def emit(nc, tc, xb, wqk, consts, yout):
    ctx_pools = []

    def pool(name, **kw):
        p = tc.alloc_tile_pool(name=name, **kw)
        ctx_pools.append(p)
        return p

    cpool = pool("cpool", bufs=1)
    ps = pool("ps", bufs=4, space="PSUM")

    cons = cpool.tile([128, CONSTS_W], F32, name="cons")
    nc.sync.dma_start(out=cons[:], in_=consts[:])
    tril = cons[:, 0:128]
    ident = cons[:, 148:276]
    negipg = cons[:, 128:144]      # [128, 16] : -gamma/(i+1)
    k1c = cons[:, 144:145]
    k2c = cons[:, 145:146]
    kbc = cons[:, 146:147]
    trilr = cpool.tile([128, 128], F32R, name="trilr")
    nc.vector.tensor_copy(out=trilr[:], in_=tril)
    ones_row = trilr[0:1, 0:128]   # tril row 0 == all ones (K=1 lhsT)
    ones_col = trilr[:, 127:128]   # tril col 127 == all ones [128, 1]

    qkp = pool("qkp", bufs=1)
    qkT = qkp.tile([128, 4, 2, 2048], F32R, name="qkT")

    # ---------------- phase A: transposes of W and x ----------------
    wtp = pool("wtp", bufs=1)
    WTq = wtp.tile([128, 4, 8, 128], F32R, name="WTq")
    WTk = wtp.tile([128, 4, 8, 128], F32R, name="WTk")
    xT = wtp.tile([128, 8, 2048], F32R, name="xT")

    ldp = pool("ldp", bufs=2)
    for p in range(4):
        for qk, WT in ((0, WTq), (1, WTk)):
            tw = ldp.tile([128, 1024], F32, name="tw", tag="tw")
            nc.sync.dma_start(out=tw[:], in_=wqk[qk * 512 + p * 128: qk * 512 + (p + 1) * 128, :])
            for cg in range(2):  # groups of 4 c-chunks
                pst = ps.tile([128, 512], F32, name="pst", tag="ps")
                for k in range(4):
                    ci = cg * 4 + k
                    nc.tensor.transpose(pst[:, k * 128:(k + 1) * 128],
                                        tw[:, ci * 128:(ci + 1) * 128], ident)
                nc.scalar.copy(out=WT[:, p, cg * 4:(cg + 1) * 4, :], in_=pst[:])
    for tt in range(NT):
        tx = ldp.tile([128, 1024], F32, name="tx", tag="tx")
        nc.sync.dma_start(out=tx[:], in_=xb[tt * 128:(tt + 1) * 128, :])
        for cg in range(2):
            pst = ps.tile([128, 512], F32, name="pstx", tag="ps")
            for k in range(4):
                ci = cg * 4 + k
                nc.tensor.transpose(pst[:, k * 128:(k + 1) * 128],
                                    tx[:, ci * 128:(ci + 1) * 128], ident)
            nc.scalar.copy(out=xT[:, cg * 4:(cg + 1) * 4, tt * 128:(tt + 1) * 128],
                           in_=pst[:].rearrange("p (a b) -> p a b", a=4))

    # ---------------- phase B: projections -> qkT ----------------
    for p in range(4):
        for qk, WT in ((0, WTq), (1, WTk)):
            for s in range(NS):
                pj = ps.tile([128, 512], F32, name="pj", tag="ps")
                for ci in range(8):
                    nc.tensor.matmul(pj[:], r(WT[:, p, ci, :]),
                                     r(xT[:, ci, s * 512:(s + 1) * 512]),
                                     start=(ci == 0), stop=(ci == 7))
                nc.vector.tensor_copy(out=qkT[:, p, qk, s * 512:(s + 1) * 512], in_=pj[:])

    # ---------------- phase B2: vones, colsum/cumsum, static ----------------
    ldp.release()
    ctx_pools.remove(ldp)
    wtp.release()
    ctx_pools.remove(wtp)
    b2 = pool("b2", bufs=1)
    b2s = pool("b2s", bufs=1)
    vones = b2.tile([128, NHC, NT, 65], F32R, name="vones")
    # strided gather of v columns: vones[p, hh, J, d] = xb[J*128+p, hh*64+d]
    nc.vector.memset(vones[:].bitcast(F32), 1.0)
    for hh in range(NHC):
        xs_view = xb[:, hh * 64:(hh + 1) * 64].rearrange("(J p) d -> p J d", p=128)
        nc.sync.dma_start(out=vones[:, hh, :, 0:64], in_=xs_view.bitcast(F32R))

    colb = b2.tile([128, 512], F32, name="colb")
    run = b2.tile([1, 512], F32R, name="run")       # exclusive prefix of tile colsums
    runs = b2.tile([1, 512], F32, name="runs")      # k2-scaled total (staging)
    static = b2.tile([128, NT, 512], F32, name="static")

    # pass 1: total colsum -> colb
    nc.vector.memset(run[:].bitcast(F32), 0.0)
    for I in range(NT):
        cp = ps.tile([1, 512], F32, name="cp", tag="cs", bufs=1)
        for hh in range(NHC):
            nc.tensor.matmul(cp[0:1, hh * 64:(hh + 1) * 64], r(ones_col),
                             r(vones[:, hh, I, 0:64]), start=True, stop=True)
        nc.vector.tensor_add(run[0:1, :], run[0:1, :], cp[0:1, :])
    nc.vector.tensor_scalar(out=runs[:], in0=run[0:1, :].bitcast(F32),
                            scalar1=cons[0:1, 145:146], scalar2=None, op0=OP.mult)
    nc.gpsimd.partition_broadcast(colb[:], runs[0:1, :])

    # pass 2: running exclusive prefix + cumsum + static
    nc.vector.memset(run[:].bitcast(F32), 0.0)
    for I in range(NT):
        cu = ps.tile([128, 512], F32, name="cu", tag="ps")
        nc.tensor.matmul(cu[:], r(ones_row), r(run[0:1, :]), start=True, stop=False)
        for hh in range(NHC):
            nc.tensor.matmul(cu[:, hh * 64:(hh + 1) * 64], r(trilr[:]),
                             r(vones[:, hh, I, 0:64]), start=False,
                             stop=(hh == NHC - 1))
        cp = ps.tile([1, 512], F32, name="cp2", tag="cs", bufs=1)
        for hh in range(NHC):
            nc.tensor.matmul(cp[0:1, hh * 64:(hh + 1) * 64], r(ones_col),
                             r(vones[:, hh, I, 0:64]), start=True, stop=True)
        nc.vector.tensor_add(run[0:1, :], run[0:1, :], cp[0:1, :])
        nc.vector.scalar_tensor_tensor(
            out=static[:, I, :].rearrange("p (h d) -> p h d", h=NHC),
            in0=vones[:, :, I, 0:64],
            scalar=k1c, in1=colb[:].rearrange("p (h d) -> p h d", h=NHC),
            op0=OP.mult, op1=OP.add)
        nc.vector.scalar_tensor_tensor(
            out=static[:, I, :], in0=cu[:], scalar=negipg[:, I:I + 1],
            in1=static[:, I, :], op0=OP.mult, op1=OP.add)

    # ---------------- phase C: attention per (head, i-strip) ----------------
    cp3 = pool("cp3", bufs=1)
    ptA = cp3.tile([128, 8, 512], F32R, name="ptA")
    ptB = cp3.tile([128, 8, 512], F32R, name="ptB")
    ysp = pool("ysp", bufs=2)

    for p in range(4):
        for half in range(2):
            hh = 2 * p + half
            base = half * 64
            qT = qkT[base:base + 64, p, 0, :]
            kT = qkT[base:base + 64, p, 1, :]
            for g in range(NS):
                nj = 4 * g + 4
                yps = ps.tile([128, 512], F32, name="yps", tag="yps", bufs=2)
                pts = []
                for J in range(nj):
                    i_off = max(0, 128 * J - 512 * g)
                    w = 512 - i_off
                    st = ps.tile([128, 512], F32, name="st", tag="ps")
                    nc.tensor.matmul(
                        st[:, i_off:512], r(kT[:, J * 128:(J + 1) * 128]),
                        r(qT[:, g * 512 + i_off:(g + 1) * 512]),
                        start=True, stop=True)
                    pt = ptA[:, J, :] if J < 8 else ptB[:, J - 8, :]
                    nc.scalar.activation(out=pt[:, i_off:512], in_=st[:, i_off:512],
                                         func=AF.Exp, scale=0.125)
                    if i_off > 0 or J == 4 * g:
                        # diagonal tile: keep j <= i only
                        nc.gpsimd.tensor_mul(pt[:, i_off:i_off + 128],
                                             pt[:, i_off:i_off + 128], tril)
                    pts.append((pt, i_off))
                for J in range(nj):
                    pt, i_off = pts[J]
                    nc.tensor.matmul(
                        yps[0:65, i_off:512], r(vones[:, hh, J, :]),
                        r(pt[:, i_off:512]),
                        start=(J == 0), stop=(J == nj - 1), skip_group_check=True)
                # evacuate y^T [65, 512], transpose back to [i, 65]
                ysb = ysp.tile([65, 512], F32, name="ysb", tag="ysb")
                nc.vector.tensor_copy(out=ysb[:], in_=yps[0:65, :])
                tp = ps.tile([128, 260], F32, name="tp", tag="tp", bufs=1)
                for k in range(4):
                    nc.tensor.transpose(tp[:, k * 65:(k + 1) * 65],
                                        ysb[:, k * 128:(k + 1) * 128], ident[0:65, 0:65])
                rc4 = ysp.tile([128, 4], F32, name="rc4", tag="rc4")
                nc.vector.reciprocal(out=rc4[:], in_=tp[:, 64:260:65])
                nc.vector.tensor_scalar(out=rc4[:], in0=rc4[:], scalar1=kbc,
                                        scalar2=None, op0=OP.mult)
                yo = ysp.tile([128, 4, 64], F32, name="yo", tag="yo")
                for k in range(4):
                    nc.vector.scalar_tensor_tensor(
                        out=yo[:, k, :], in0=tp[:, k * 65:k * 65 + 64],
                        scalar=rc4[:, k:k + 1],
                        in1=static[:, 4 * g + k, hh * 64:(hh + 1) * 64],
                        op0=OP.mult, op1=OP.add)
                nc.sync.dma_start(
                    out=yout[g * 512:(g + 1) * 512, hh * 64:(hh + 1) * 64]
                    .rearrange("(k p) d -> p k d", p=128),
                    in_=yo[:])

    for p in reversed(ctx_pools):
        p.release()


def build_nc():
    if "nc" in _NC_CACHE:
        return _NC_CACHE["nc"]
    nc = bacc.Bacc("TRN2", target_bir_lowering=False)
    xb = nc.declare_dram_parameter("xb", [T, C], F32, isOutput=False)
    wqk = nc.declare_dram_parameter("wqk", [C, C], F32, isOutput=False)
    consts = nc.declare_dram_parameter("consts", [128, CONSTS_W], F32, isOutput=False)
    yout = nc.declare_dram_parameter("yout", [T, 512], F32, isOutput=True)
    with tile.TileContext(nc) as tc:
        emit(nc, tc, xb, wqk, consts, yout)
    nc.compile()
    _NC_CACHE["nc"] = nc
    return nc


def make_consts(alpha, beta, gamma):
    D = math.e + T - 1
    k1 = alpha * (math.e - 1.0) / D
    k2 = alpha / D
    cons = np.zeros((128, CONSTS_W), dtype=np.float32)
    jj = np.arange(128)
    cons[:, 0:128] = (jj[:, None] <= jj[None, :]).astype(np.float32)  # tril mask
    for I in range(16):
        cons[:, 128 + I] = -gamma / (128.0 * I + jj + 1.0)
    cons[:, 144] = k1
    cons[:, 145] = k2
    cons[:, 146] = beta
    cons[:, 148:276] = np.eye(128, dtype=np.float32)
    return cons


def kernel(x, w_attn, alpha, beta, gamma, _trace=False):
    x = np.asarray(x, dtype=np.float32)
    w_attn = np.asarray(w_attn, dtype=np.float32)
    alpha = float(np.asarray(alpha))
    beta = float(np.asarray(beta))
    gamma = float(np.asarray(gamma))

    nc = build_nc()
    cons = make_consts(alpha, beta, gamma)
    in_maps = []
    for c in range(N_CORES):
        b, h0 = c // 2, (c % 2) * 8
        wqk = np.concatenate(
            [w_attn[h0 * 64: h0 * 64 + 512], w_attn[C + h0 * 64: C + h0 * 64 + 512]], axis=0)
        # rotate columns of x and w so this core's v-block sits at columns 0:512
        # (the projection q,k = x @ w.T is invariant to a consistent column roll)
        c0 = h0 * 64
        xb_r = np.roll(x[b], -c0, axis=1)
        wqk_r = np.roll(wqk, -c0, axis=1)
        in_maps.append({"xb": np.ascontiguousarray(xb_r),
                        "wqk": np.ascontiguousarray(wqk_r), "consts": cons})
    res = run_bass_kernel_spmd(nc, in_maps, list(range(N_CORES)), trace=_trace)
    y = np.empty((B, T, C), dtype=np.float32)
    for c in range(N_CORES):
        b, h0 = c // 2, (c % 2) * 8
        y[b, :, h0 * 64: h0 * 64 + 512] = res.results[c]["yout"]
    if _trace:
        kernel.last_exec_time_ns = res.exec_time_ns
    return y
